# revision 10
# baseline (speedup 1.0000x reference)
"""AvgPool2d(16x16, stride 1) with replicate-padded output — hybrid
Bass/Tile kernel for 8 Trainium2 NeuronCores, fp16 I/O.

out[hp, wp] = (1/256) * sum_{16x16 box} x[clamp-window] per (n,c) plane;
256 planes total, 32 per core, data-parallel, no comms.

v2 structure (from v1 trace: DVE 136.8us busy = scans 107.7 + sems 23.2;
ACT 131.2 = evacs 99.3 + store-issue 19.1; exec 148us):

SCAN planes (22):
  ONE fused tensor_tensor_scan per plane over a [128, 2096] stream of
  4x(16 zeros + 512 cols): the 16-zero gap drains the window state to
  exactly 0 between chunks (s_t = s_{t-1} + d0[t] - d1[t] with d1 = d0
  shifted 16 keeps only the last-16 window, so 16 zero-adds while
  subtracting the previous chunk tail flushes it). fp16 out (no DVE 2x
  for scan — measured 1125ns either way — but fp32 internal state, so
  no drift). H-window + H-replicate-pad as a rolled banded fp16 matmul
  (8 MMs of N=500), evac f32 PSUM -> *1/256 -> fp16 on ACT.

PE planes (10, zero DVE, no on-chip transpose — replaces v1's
H-MM/16-transpose/W-MM pipeline, cutting its ACT cost ~6.2->3.9us and
PE stream ~7.7->5.1us):
  Host pre-transposes + rolls those planes. Step A: y = x.B_w via
  lhsT = xT chunks (data as stationary!), rhs = band chunks, N=512,
  16 MMs accumulating 4 chunks per 128-row group; evac y -> fp16.
  Step B (staggered to p+2): out = B_h^T.y, the same 8-MM rolled band
  as scan planes, evac *1/256 -> fp16.
  ONE [512,512] fp16 scale-1.0 clamped+rolled band matrix serves all
  three MM stages (weights 0.5MiB vs v1's 2.6MiB).

Stores issue from GpSimd (SWDGE): ACT sequencer spent 19.1us issuing
them; SP-issued stores stall the load stream (v1: +13us).
Loads on Sync (HWDGE), pinned in a dep-chain to keep queue phase stable.
Only the FINAL plane's evacs go to DVE (tail latency only).
24-MM warm-up bridge on the first preloaded tile keeps the PE HAM
window from re-throttling before the first real matmul (v1: bimodal
148/173us without it).

HOST does all swizzles: fp16 downcast, +7 rolls, zero gaps, transpose
for PE planes, fp32 upcast, W replicate-pad (for PE planes the on-chip
clamped band already wrote identical values; the pad is a no-op).
"""
import numpy as np
from contextlib import ExitStack

import concourse.bass as bass
import concourse.bacc as bacc
import concourse.tile as tile
from concourse import mybir
from concourse.bass_utils import run_bass_kernel_spmd
from concourse.tile import add_dep_helper

NCORES = 8
N, C, H, W = 4, 64, 512, 512
K = 16
NW = H - K + 1         # 497 valid box positions per axis
PAD_T = (H - NW) // 2  # 7 (same for W)
PLANES = (N * C) // NCORES  # 32 planes per core
NCH = H // 128         # 4 row-chunks of 128
SEG = W + K            # 528: 16-col zero gap + 512 data per chunk segment
SCAN_LEN = NCH * SEG   # 2112 fused-scan tile width
PE_COLS = NCH * W      # 2048 PE-plane tile width

# planes handled by the all-PE (host-transposed) pipeline; rest scan.
# Spaced >=2 apart; stage B of plane p is emitted at step p+2; last PE
# plane at 28 so steps 29-31 are pure scan (no stage tail after loop).
PE_SET = frozenset((1, 3, 5, 7, 9, 11, 13, 15, 17, 19, 20, 22, 24, 26, 28))


def _band_matrix(scale: float, roll: bool) -> np.ndarray:
    """BT[h, hp] = scale on the clamped band; lhsT layout for out = BT.T @ rhs.

    roll=True: rolled by +PAD_T along h so each 128-row chunk c covers rows
    [128c-7, 128c+121) and every 128-row output group needs exactly TWO
    contraction chunks (wrap-around lands in chunk 0, matching the host's
    +7 data roll)."""
    bt = np.zeros((H, H), np.float32)
    for hp in range(H):
        lo = min(max(hp - PAD_T, 0), H - K)
        bt[lo:lo + K, hp] = scale
    return np.roll(bt, PAD_T, axis=0) if roll else bt


def _k_chunks(bt: np.ndarray) -> list[list[int]]:
    ks = []
    for m in range(NCH):
        ks.append([c for c in range(NCH)
                   if np.any(bt[128 * c:128 * (c + 1), 128 * m:128 * (m + 1)])])
    return ks


def _build_program(planes: int = PLANES):
    f32 = mybir.dt.float32
    f16 = mybir.dt.float16
    ks_roll = _k_chunks(_band_matrix(1.0, True))
    inv = 1.0 / (K * K)

    nc = bacc.Bacc("TRN2", target_bir_lowering=False, debug=False,
                   num_devices=NCORES, num_swdge_queues=4)
    x_ap = nc.dram_tensor("x", [planes, 128, SCAN_LEN], f16,
                          kind="ExternalInput").ap()
    band_ap = nc.dram_tensor("band", [128, NCH, H], f16,
                             kind="ExternalInput").ap()
    o_ap = nc.dram_tensor("out", [planes, 128, NCH, W], f16,
                          kind="ExternalOutput").ap()

    with tile.TileContext(nc) as tc, ExitStack() as ctx:
        wpool = ctx.enter_context(tc.tile_pool(name="wt", bufs=1))
        xpool = ctx.enter_context(tc.tile_pool(name="xt", bufs=4))
        xtpool = ctx.enter_context(tc.tile_pool(name="xT", bufs=3))
        bwpool = ctx.enter_context(tc.tile_pool(name="bw", bufs=3))
        ypool = ctx.enter_context(tc.tile_pool(name="yt", bufs=3))
        opool = ctx.enter_context(tc.tile_pool(name="osb", bufs=6))
        # 8 PSUM banks: ps_mm (scan H-MM, warm-up, PE step B) 2x2 banks,
        # ps_s1 (PE step A) 2x2 banks.
        ps_mm = ctx.enter_context(tc.tile_pool(name="psmm", bufs=2,
                                               space="PSUM"))
        ps_s1 = ctx.enter_context(tc.tile_pool(name="pss1", bufs=1,
                                               space="PSUM"))

        # Ordering-only pins keep the HWDGE round-robin phase stable-ish.
        dma_chain = []

        def chain(inst):
            if dma_chain:
                add_dep_helper(inst.ins, dma_chain[-1].ins, sync=False,
                               reason="pin HWDGE round-robin phase")
            dma_chain.append(inst)

        # Hoist the first two plane loads AHEAD of the weight DMA so the
        # DVE scan starts immediately.
        preloaded = {}
        pre0 = wpool.tile([128, SCAN_LEN], f16, tag="xt_pre0")
        chain(nc.sync.dma_start(pre0[:, :], x_ap[0]))
        preloaded[0] = pre0
        if planes > 1:
            pre1 = wpool.tile([128, PE_COLS], f16, tag="xt_pre1")
            chain(nc.sync.dma_start(pre1[:, :], x_ap[1, :, 0:PE_COLS]))
            preloaded[1] = pre1
        band = wpool.tile([128, NCH, H], f16, tag="band")
        chain(nc.sync.dma_start(band[:, :, :], band_ap))

        # HAM warm-up bridge: dummy fp16 MMs on the preloaded tile keep
        # the PE busy from xt0 arrival until the first real matmul.
        pt_warm = ps_mm.tile([128, 2, W], f32, tag="pt")
        for _ in range(24):
            nc.tensor.matmul(pt_warm[:, 0, :],
                             lhsT=pre0[:, K:K + 128],
                             rhs=pre0[:, K:K + W],
                             start=True, stop=True, skip_group_check=True)

        # PE-path stage B (H-band MM + store) is STAGGERED to step p+2 so
        # the interleaved scan-plane matmuls cover the y-evac latency in
        # the in-order PE instruction stream.
        pending = []

        def emit_B(st):
            y16 = st["y"]
            osb = opool.tile([128, NCH, W], f16)
            for half in range(2):
                pt = ps_mm.tile([128, 2, W], f32, tag="pt")
                for mi in (2 * half, 2 * half + 1):
                    ks = ks_roll[mi]
                    for i, c in enumerate(ks):
                        nc.tensor.matmul(
                            pt[:, mi - 2 * half, :],
                            lhsT=band[:, c, 128 * mi:128 * (mi + 1)],
                            rhs=y16[:, c, :],
                            start=(i == 0),
                            stop=(i == len(ks) - 1),
                        )
                with nc.allow_low_precision("fp16 output store"):
                    nc.scalar.mul(osb[:, 2 * half:2 * half + 2, :],
                                  pt[:, :, :], inv)
            # stores on GpSimd/SWDGE: ACT spent 19us issuing them, and
            # SP-issued stores stall the HWDGE load stream (v1: +13us).
            nc.gpsimd.dma_start(o_ap[st["p"]], osb[:, :, :])

        def advance(step):
            for st in list(pending):
                if st["b_due"] == step:
                    emit_B(st)
                    pending.remove(st)

        for p in range(planes):
            advance(p)
            if p in preloaded:
                xt = preloaded[p]
            elif p in PE_SET:
                xt = xtpool.tile([128, PE_COLS], f16)
                chain(nc.sync.dma_start(xt[:, :], x_ap[p, :, 0:PE_COLS]))
            else:
                xt = xpool.tile([128, SCAN_LEN], f16)
                chain(nc.sync.dma_start(xt[:, :], x_ap[p]))

            if p not in PE_SET:
                # ---------- scan pipeline ----------
                b = bwpool.tile([128, SCAN_LEN], f16)
                with nc.allow_low_precision("f16 bw; fp32 scan state"):
                    # state_t = (d0[t] + state) - d1[t] with d1 = d0
                    # shifted 16: col t holds the 16-window sum ending at
                    # d0[t]; the 16-zero gaps flush state to 0 between
                    # the four chunk segments.
                    nc.vector.tensor_tensor_scan(
                        out=b[:, K:SCAN_LEN],
                        data0=xt[:, K:SCAN_LEN],
                        data1=xt[:, 0:SCAN_LEN - K],
                        initial=0.0,
                        op0=mybir.AluOpType.add,
                        op1=mybir.AluOpType.subtract,
                    )
                osb = opool.tile([128, NCH, W], f16)
                for half in range(2):
                    pt = ps_mm.tile([128, 2, W], f32, tag="pt")
                    for mi in (2 * half, 2 * half + 1):
                        ks = ks_roll[mi]
                        for i, c in enumerate(ks):
                            nc.tensor.matmul(
                                pt[:, mi - 2 * half, PAD_T - 3:PAD_T + NW],
                                lhsT=band[:, c, 128 * mi:128 * (mi + 1)],
                                rhs=b[:, SEG * c + K + 12:SEG * c + SEG],
                                start=(i == 0),
                                stop=(i == len(ks) - 1),
                            )
                    with nc.allow_low_precision("fp16 output store"):
                        # only the LAST TWO planes' evacs go to DVE:
                        # earlier ones would sit ahead of later scans in
                        # DVE's in-order stream and delay the tail
                        if p >= planes - 2:
                            nc.vector.tensor_scalar_mul(
                                osb[:, 2 * half:2 * half + 2,
                                    PAD_T:PAD_T + NW],
                                pt[:, :, PAD_T:PAD_T + NW], inv)
                        else:
                            nc.scalar.mul(
                                osb[:, 2 * half:2 * half + 2,
                                    PAD_T:PAD_T + NW],
                                pt[:, :, PAD_T:PAD_T + NW], inv)
                nc.gpsimd.dma_start(o_ap[p], osb[:, :, :])
            else:
                # ---------- all-PE pipeline, step A: y = x.B_w ----------
                # lhsT = host-transposed data chunks (stationary), rhs =
                # band chunks full width; the band's clamp covers the W
                # replicate-pad, its wrap-around rows live in chunk 0.
                y16 = ypool.tile([128, NCH, W], f16)
                ps1 = ps_s1.tile([128, NCH, W], f32, tag="s1")
                for m in range(NCH):
                    for c in range(NCH):
                        # band chunk 0 spans all 512 cols (clamp + roll
                        # wrap-around) and initializes the full
                        # accumulator; chunks 1-3 only touch a 144-col
                        # window (the stop flag rides the last window).
                        if c > 0:
                            cols = slice(128 * c - 16, 128 * c + 128)
                        else:
                            cols = slice(0, W)
                        nc.tensor.matmul(
                            ps1[:, m, cols],
                            lhsT=xt[:, W * c + 128 * m:
                                    W * c + 128 * m + 128],
                            rhs=band[:, c, cols],
                            start=(c == 0),
                            stop=(c == NCH - 1),
                        )
                with nc.allow_low_precision("f16 y intermediate"):
                    # single wide evac: one ACTIVATE over all 4 banks
                    nc.scalar.copy(y16[:, :, :], ps1[:, :, :])
                pending.append({"p": p, "y": y16, "b_due": p + 2})
        for extra in range(planes, planes + 3):
            advance(extra)

    nc.compile()
    return nc


_NC_CACHE = {}


def _get_nc(planes: int = PLANES):
    if planes not in _NC_CACHE:
        _NC_CACHE[planes] = _build_program(planes)
    return _NC_CACHE[planes]


def _swizzle_in(planes_all: np.ndarray) -> np.ndarray:
    """[32,512,512] fp32 -> [32,128,SCAN_LEN] fp16.

    scan planes: rows rolled +7; per chunk segment [16 zeros | 512 data];
    partition q, segment c holds rolled-plane row 128c+q.
    PE planes: plane rolled +7 on BOTH axes then transposed; cols
    [0:2048) = [q, (c, h)] with partition q, chunk c = w rows 128c+q."""
    p = planes_all.shape[0]
    xin = np.zeros((p, 128, NCH, SEG), np.float16)
    s_idx = [i for i in range(p) if i not in PE_SET]
    xr = np.roll(planes_all[s_idx], PAD_T, axis=1)
    xin[s_idx, :, :, K:] = np.ascontiguousarray(
        xr.reshape(-1, NCH, 128, W).transpose(0, 2, 1, 3))
    flat = xin.reshape(p, 128, SCAN_LEN)
    p_idx = [i for i in range(p) if i in PE_SET]
    if p_idx:
        xt2 = np.roll(planes_all[p_idx], PAD_T,
                      axis=(1, 2)).transpose(0, 2, 1)  # [b, w_r, h_r]
        flat[p_idx, :, 0:PE_COLS] = (
            xt2.reshape(-1, NCH, 128, W).transpose(0, 2, 1, 3)
            .reshape(-1, 128, PE_COLS))
    return flat


def _unswizzle_out(oswz: np.ndarray) -> np.ndarray:
    """[P,128,NCH,512] fp16 -> [P,512,512] fp32; W replicate-pad (no-op
    for PE planes whose clamped band already wrote the edge values)."""
    o = oswz.astype(np.float32)
    out = o.transpose(0, 2, 1, 3).reshape(-1, H, W)
    out[:, :, 0:PAD_T] = out[:, :, PAD_T:PAD_T + 1]
    out[:, :, PAD_T + NW:] = out[:, :, PAD_T + NW - 1:PAD_T + NW]
    return out


def run_sharded(x: np.ndarray, trace: bool = False, trace_cores=None, **kw):
    """x: (N, C, H, W) fp32 -> (out (N,C,H,W) fp32, BassKernelResults)."""
    nc = _get_nc()
    planes_all = np.ascontiguousarray(x.reshape(N * C, H, W), dtype=np.float32)
    band = _band_matrix(1.0, True).astype(np.float16)
    band_in = np.ascontiguousarray(
        band.reshape(NCH, 128, H).transpose(1, 0, 2))
    in_maps = [
        {"x": _swizzle_in(planes_all[i * PLANES:(i + 1) * PLANES]),
         "band": band_in}
        for i in range(NCORES)
    ]
    r = run_bass_kernel_spmd(nc, in_maps, list(range(NCORES)),
                             trace=trace, trace_cores=trace_cores, **kw)
    out = np.concatenate(
        [_unswizzle_out(r.results[i]["out"]) for i in range(NCORES)], axis=0)
    return out.reshape(N, C, H, W), r


def kernel(x: np.ndarray) -> np.ndarray:
    out, _ = run_sharded(np.asarray(x))
    return out


if __name__ == "__main__":
    # quick compile-only probe with a reduced plane count
    import sys
    import tempfile
    from concourse.bass_utils import compile_bir_kernel

    planes = int(sys.argv[1]) if len(sys.argv) > 1 else 8
    nc = _build_program(planes)
    d = tempfile.mkdtemp()
    print(f"compiling {planes}-plane program to {d} ...")
    neff = compile_bir_kernel(nc.to_json_bytes(), d, neff_name="probe.neff")
    print(f"COMPILE OK: {neff}")


# revision 12
# speedup vs baseline: 1.0529x; 1.0529x over previous
"""AvgPool2d(16x16, stride 1) with replicate-padded output — hybrid
Bass/Tile kernel for 8 Trainium2 NeuronCores, fp16 I/O.

out[hp, wp] = (1/256) * sum_{16x16 box} x[clamp-window] per (n,c) plane;
256 planes total, 32 per core, data-parallel, no comms.

v2 structure (from v1 trace: DVE 136.8us busy = scans 107.7 + sems 23.2;
ACT 131.2 = evacs 99.3 + store-issue 19.1; exec 148us):

SCAN planes (22):
  ONE fused tensor_tensor_scan per plane over a [128, 2096] stream of
  4x(16 zeros + 512 cols): the 16-zero gap drains the window state to
  exactly 0 between chunks (s_t = s_{t-1} + d0[t] - d1[t] with d1 = d0
  shifted 16 keeps only the last-16 window, so 16 zero-adds while
  subtracting the previous chunk tail flushes it). fp16 out (no DVE 2x
  for scan — measured 1125ns either way — but fp32 internal state, so
  no drift). H-window + H-replicate-pad as a rolled banded fp16 matmul
  (8 MMs of N=500), evac f32 PSUM -> *1/256 -> fp16 on ACT.

PE planes (10, zero DVE, no on-chip transpose — replaces v1's
H-MM/16-transpose/W-MM pipeline, cutting its ACT cost ~6.2->3.9us and
PE stream ~7.7->5.1us):
  Host pre-transposes + rolls those planes. Step A: y = x.B_w via
  lhsT = xT chunks (data as stationary!), rhs = band chunks, N=512,
  16 MMs accumulating 4 chunks per 128-row group; evac y -> fp16.
  Step B (staggered to p+2): out = B_h^T.y, the same 8-MM rolled band
  as scan planes, evac *1/256 -> fp16.
  ONE [512,512] fp16 scale-1.0 clamped+rolled band matrix serves all
  three MM stages (weights 0.5MiB vs v1's 2.6MiB).

Stores issue from GpSimd (SWDGE): ACT sequencer spent 19.1us issuing
them; SP-issued stores stall the load stream (v1: +13us).
Loads on Sync (HWDGE), pinned in a dep-chain to keep queue phase stable.
Only the FINAL plane's evacs go to DVE (tail latency only).
24-MM warm-up bridge on the first preloaded tile keeps the PE HAM
window from re-throttling before the first real matmul (v1: bimodal
148/173us without it).

HOST does all swizzles: fp16 downcast, +7 rolls, zero gaps, transpose
for PE planes, fp32 upcast, W replicate-pad (for PE planes the on-chip
clamped band already wrote identical values; the pad is a no-op).
"""
import numpy as np
from contextlib import ExitStack

import concourse.bass as bass
import concourse.bacc as bacc
import concourse.tile as tile
from concourse import mybir
from concourse.bass_utils import run_bass_kernel_spmd
from concourse.tile import add_dep_helper

NCORES = 8
N, C, H, W = 4, 64, 512, 512
K = 16
NW = H - K + 1         # 497 valid box positions per axis
PAD_T = (H - NW) // 2  # 7 (same for W)
PLANES = (N * C) // NCORES  # 32 planes per core
NCH = H // 128         # 4 row-chunks of 128
SEG = W + K            # 528: 16-col zero gap + 512 data per chunk segment
SCAN_LEN = NCH * SEG   # 2112 fused-scan tile width
PE_COLS = NCH * W      # 2048 PE-plane tile width

# planes handled by the all-PE (host-transposed) pipeline; rest scan.
# Spaced >=2 apart; stage B of plane p is emitted at step p+2; last PE
# plane at 28 so steps 29-31 are pure scan (no stage tail after loop).
PE_SET = frozenset((1, 3, 5, 7, 9, 11, 13, 15, 17, 19, 20, 22, 24, 26, 28))


def _band_matrix(scale: float, roll: bool) -> np.ndarray:
    """BT[h, hp] = scale on the clamped band; lhsT layout for out = BT.T @ rhs.

    roll=True: rolled by +PAD_T along h so each 128-row chunk c covers rows
    [128c-7, 128c+121) and every 128-row output group needs exactly TWO
    contraction chunks (wrap-around lands in chunk 0, matching the host's
    +7 data roll)."""
    bt = np.zeros((H, H), np.float32)
    for hp in range(H):
        lo = min(max(hp - PAD_T, 0), H - K)
        bt[lo:lo + K, hp] = scale
    return np.roll(bt, PAD_T, axis=0) if roll else bt


def _k_chunks(bt: np.ndarray) -> list[list[int]]:
    ks = []
    for m in range(NCH):
        ks.append([c for c in range(NCH)
                   if np.any(bt[128 * c:128 * (c + 1), 128 * m:128 * (m + 1)])])
    return ks


def _build_program(planes: int = PLANES):
    f32 = mybir.dt.float32
    f16 = mybir.dt.float16
    ks_roll = _k_chunks(_band_matrix(1.0, True))
    inv = 1.0 / (K * K)

    nc = bacc.Bacc("TRN2", target_bir_lowering=False, debug=False,
                   num_devices=NCORES, num_swdge_queues=4)
    x_ap = nc.dram_tensor("x", [planes, 128, SCAN_LEN], f16,
                          kind="ExternalInput").ap()
    band_ap = nc.dram_tensor("band", [128, NCH, H], f16,
                             kind="ExternalInput").ap()
    o_ap = nc.dram_tensor("out", [planes, 128, NCH, W], f16,
                          kind="ExternalOutput").ap()

    with tile.TileContext(nc) as tc, ExitStack() as ctx:
        wpool = ctx.enter_context(tc.tile_pool(name="wt", bufs=1))
        xpool = ctx.enter_context(tc.tile_pool(name="xt", bufs=4))
        xtpool = ctx.enter_context(tc.tile_pool(name="xT", bufs=3))
        bwpool = ctx.enter_context(tc.tile_pool(name="bw", bufs=3))
        ypool = ctx.enter_context(tc.tile_pool(name="yt", bufs=3))
        opool = ctx.enter_context(tc.tile_pool(name="osb", bufs=6))
        # 8 PSUM banks: ps_mm (scan H-MM, warm-up, PE step B) 2x2 banks,
        # ps_s1 (PE step A) 2x2 banks.
        ps_mm = ctx.enter_context(tc.tile_pool(name="psmm", bufs=2,
                                               space="PSUM"))
        ps_s1 = ctx.enter_context(tc.tile_pool(name="pss1", bufs=2,
                                               space="PSUM"))

        # Ordering-only pins keep the HWDGE round-robin phase stable-ish.
        dma_chain = []

        def chain(inst):
            if dma_chain:
                add_dep_helper(inst.ins, dma_chain[-1].ins, sync=False,
                               reason="pin HWDGE round-robin phase")
            dma_chain.append(inst)

        # Hoist the first two plane loads AHEAD of the weight DMA so the
        # DVE scan starts immediately.
        preloaded = {}
        pre0 = wpool.tile([128, SCAN_LEN], f16, tag="xt_pre0")
        chain(nc.sync.dma_start(pre0[:, :], x_ap[0]))
        preloaded[0] = pre0
        if planes > 1:
            pre1 = wpool.tile([128, PE_COLS], f16, tag="xt_pre1")
            chain(nc.sync.dma_start(pre1[:, :], x_ap[1, :, 0:PE_COLS]))
            preloaded[1] = pre1
        band = wpool.tile([128, NCH, H], f16, tag="band")
        chain(nc.sync.dma_start(band[:, :, :], band_ap))

        # HAM warm-up bridge: dummy fp16 MMs on the preloaded tile keep
        # the PE busy from xt0 arrival until the first real matmul.
        pt_warm = ps_mm.tile([128, 2, W], f32, tag="pt")
        for _ in range(24):
            nc.tensor.matmul(pt_warm[:, 0, :],
                             lhsT=pre0[:, K:K + 128],
                             rhs=pre0[:, K:K + W],
                             start=True, stop=True, skip_group_check=True)

        # PE-path stage B (H-band MM + store) is STAGGERED to step p+2 so
        # the interleaved scan-plane matmuls cover the y-evac latency in
        # the in-order PE instruction stream.
        pending = []

        def emit_B(st):
            y16 = st["y"]
            osb = opool.tile([128, NCH, W], f16)
            for half in range(2):
                pt = ps_mm.tile([128, 2, W], f32, tag="pt")
                for mi in (2 * half, 2 * half + 1):
                    ks = ks_roll[mi]
                    for i, c in enumerate(ks):
                        nc.tensor.matmul(
                            pt[:, mi - 2 * half, :],
                            lhsT=band[:, c, 128 * mi:128 * (mi + 1)],
                            rhs=y16[:, c, :],
                            start=(i == 0),
                            stop=(i == len(ks) - 1),
                        )
                with nc.allow_low_precision("fp16 output store"):
                    nc.scalar.mul(osb[:, 2 * half:2 * half + 2, :],
                                  pt[:, :, :], inv)
            # stores on GpSimd/SWDGE: ACT spent 19us issuing them, and
            # SP-issued stores stall the HWDGE load stream (v1: +13us).
            nc.gpsimd.dma_start(o_ap[st["p"]], osb[:, :, :])

        def advance(step):
            for st in list(pending):
                if st["b_due"] == step:
                    emit_B(st)
                    pending.remove(st)

        for p in range(planes):
            advance(p)
            if p in preloaded:
                xt = preloaded[p]
            elif p in PE_SET:
                xt = xtpool.tile([128, PE_COLS], f16)
                chain(nc.sync.dma_start(xt[:, :], x_ap[p, :, 0:PE_COLS]))
            else:
                xt = xpool.tile([128, SCAN_LEN], f16)
                chain(nc.sync.dma_start(xt[:, :], x_ap[p]))

            if p not in PE_SET:
                # ---------- scan pipeline ----------
                b = bwpool.tile([128, SCAN_LEN], f16)
                with nc.allow_low_precision("f16 bw; fp32 scan state"):
                    # state_t = (d0[t] + state) - d1[t] with d1 = d0
                    # shifted 16: col t holds the 16-window sum ending at
                    # d0[t]; the 16-zero gaps flush state to 0 between
                    # the four chunk segments.
                    nc.vector.tensor_tensor_scan(
                        out=b[:, K:SCAN_LEN],
                        data0=xt[:, K:SCAN_LEN],
                        data1=xt[:, 0:SCAN_LEN - K],
                        initial=0.0,
                        op0=mybir.AluOpType.add,
                        op1=mybir.AluOpType.subtract,
                    )
                osb = opool.tile([128, NCH, W], f16)
                for half in range(2):
                    pt = ps_mm.tile([128, 2, W], f32, tag="pt")
                    for mi in (2 * half, 2 * half + 1):
                        ks = ks_roll[mi]
                        for i, c in enumerate(ks):
                            nc.tensor.matmul(
                                pt[:, mi - 2 * half, PAD_T - 3:PAD_T + NW],
                                lhsT=band[:, c, 128 * mi:128 * (mi + 1)],
                                rhs=b[:, SEG * c + K + 12:SEG * c + SEG],
                                start=(i == 0),
                                stop=(i == len(ks) - 1),
                            )
                    with nc.allow_low_precision("fp16 output store"):
                        # only the LAST TWO planes' evacs go to DVE:
                        # earlier ones would sit ahead of later scans in
                        # DVE's in-order stream and delay the tail
                        if p >= planes - 2:
                            nc.vector.tensor_scalar_mul(
                                osb[:, 2 * half:2 * half + 2,
                                    PAD_T:PAD_T + NW],
                                pt[:, :, PAD_T:PAD_T + NW], inv)
                        else:
                            nc.scalar.mul(
                                osb[:, 2 * half:2 * half + 2,
                                    PAD_T:PAD_T + NW],
                                pt[:, :, PAD_T:PAD_T + NW], inv)
                nc.gpsimd.dma_start(o_ap[p], osb[:, :, :])
            else:
                # ---------- all-PE pipeline, step A: y = x.B_w ----------
                # lhsT = host-transposed data chunks (stationary), rhs =
                # band chunks full width; the band's clamp covers the W
                # replicate-pad, its wrap-around rows live in chunk 0.
                y16 = ypool.tile([128, NCH, W], f16)
                for half in range(2):
                    ps1 = ps_s1.tile([128, 2, W], f32, tag="s1")
                    for m in (2 * half, 2 * half + 1):
                        for c in range(NCH):
                            # band chunk 0 spans all 512 cols (clamp +
                            # roll wrap-around) and initializes the full
                            # accumulator; chunks 1-3 only touch a
                            # 144-col window (stop rides the last one —
                            # partial-coverage stop measured legal).
                            if c > 0:
                                cols = slice(128 * c - 16, 128 * c + 128)
                            else:
                                cols = slice(0, W)
                            nc.tensor.matmul(
                                ps1[:, m - 2 * half, cols],
                                lhsT=xt[:, W * c + 128 * m:
                                        W * c + 128 * m + 128],
                                rhs=band[:, c, cols],
                                start=(c == 0),
                                stop=(c == NCH - 1),
                            )
                    with nc.allow_low_precision("f16 y intermediate"):
                        nc.scalar.copy(y16[:, 2 * half:2 * half + 2, :],
                                       ps1[:, :, :])
                pending.append({"p": p, "y": y16, "b_due": p + 2})
        for extra in range(planes, planes + 3):
            advance(extra)

    nc.compile()
    return nc


_NC_CACHE = {}


def _get_nc(planes: int = PLANES):
    if planes not in _NC_CACHE:
        _NC_CACHE[planes] = _build_program(planes)
    return _NC_CACHE[planes]


def _swizzle_in(planes_all: np.ndarray) -> np.ndarray:
    """[32,512,512] fp32 -> [32,128,SCAN_LEN] fp16.

    scan planes: rows rolled +7; per chunk segment [16 zeros | 512 data];
    partition q, segment c holds rolled-plane row 128c+q.
    PE planes: plane rolled +7 on BOTH axes then transposed; cols
    [0:2048) = [q, (c, h)] with partition q, chunk c = w rows 128c+q."""
    p = planes_all.shape[0]
    xin = np.zeros((p, 128, NCH, SEG), np.float16)
    s_idx = [i for i in range(p) if i not in PE_SET]
    xr = np.roll(planes_all[s_idx], PAD_T, axis=1)
    xin[s_idx, :, :, K:] = np.ascontiguousarray(
        xr.reshape(-1, NCH, 128, W).transpose(0, 2, 1, 3))
    flat = xin.reshape(p, 128, SCAN_LEN)
    p_idx = [i for i in range(p) if i in PE_SET]
    if p_idx:
        xt2 = np.roll(planes_all[p_idx], PAD_T,
                      axis=(1, 2)).transpose(0, 2, 1)  # [b, w_r, h_r]
        flat[p_idx, :, 0:PE_COLS] = (
            xt2.reshape(-1, NCH, 128, W).transpose(0, 2, 1, 3)
            .reshape(-1, 128, PE_COLS))
    return flat


def _unswizzle_out(oswz: np.ndarray) -> np.ndarray:
    """[P,128,NCH,512] fp16 -> [P,512,512] fp32; W replicate-pad (no-op
    for PE planes whose clamped band already wrote the edge values)."""
    o = oswz.astype(np.float32)
    out = o.transpose(0, 2, 1, 3).reshape(-1, H, W)
    out[:, :, 0:PAD_T] = out[:, :, PAD_T:PAD_T + 1]
    out[:, :, PAD_T + NW:] = out[:, :, PAD_T + NW - 1:PAD_T + NW]
    return out


def run_sharded(x: np.ndarray, trace: bool = False, trace_cores=None, **kw):
    """x: (N, C, H, W) fp32 -> (out (N,C,H,W) fp32, BassKernelResults)."""
    nc = _get_nc()
    planes_all = np.ascontiguousarray(x.reshape(N * C, H, W), dtype=np.float32)
    band = _band_matrix(1.0, True).astype(np.float16)
    band_in = np.ascontiguousarray(
        band.reshape(NCH, 128, H).transpose(1, 0, 2))
    in_maps = [
        {"x": _swizzle_in(planes_all[i * PLANES:(i + 1) * PLANES]),
         "band": band_in}
        for i in range(NCORES)
    ]
    r = run_bass_kernel_spmd(nc, in_maps, list(range(NCORES)),
                             trace=trace, trace_cores=trace_cores, **kw)
    out = np.concatenate(
        [_unswizzle_out(r.results[i]["out"]) for i in range(NCORES)], axis=0)
    return out.reshape(N, C, H, W), r


def kernel(x: np.ndarray) -> np.ndarray:
    out, _ = run_sharded(np.asarray(x))
    return out


if __name__ == "__main__":
    # quick compile-only probe with a reduced plane count
    import sys
    import tempfile
    from concourse.bass_utils import compile_bir_kernel

    planes = int(sys.argv[1]) if len(sys.argv) > 1 else 8
    nc = _build_program(planes)
    d = tempfile.mkdtemp()
    print(f"compiling {planes}-plane program to {d} ...")
    neff = compile_bir_kernel(nc.to_json_bytes(), d, neff_name="probe.neff")
    print(f"COMPILE OK: {neff}")
